# revision 9
# baseline (speedup 1.0000x reference)
"""Trainium2 Bass kernel for a dense transformer block (attention + LoRA +
MLP + proj), data-parallel over batch across 8 NeuronCores.

Contract: kernel(**inputs) takes the FULL unsharded inputs (numpy arrays,
keys as in reference.setup_inputs()) and returns the FULL [8, 512, 1024]
fp32 output.

Design (per core, one batch element):
  - Everything flows channel-major ("transposed"): activations are [C, S]
    tiles with channels on SBUF partitions.  All weights are then used in
    their natural [C_in, C_out] layout (as stationary lhsT slices for
    channel-major outputs, as moving rhs for the token-major v), and the
    only transposes in the whole pipeline happen on the host (x -> x.T in,
    out.T -> out).
  - Attention runs keys-on-partitions (attnT = K q^T per head).  The key
    mask folds into the softmax exp as a per-partition bias (0 / -50), the
    1/sqrt(hd) scale folds into the exp's scale, and the softmax
    denominator comes free as a ones-column appended to v in the PV matmul
    (M=65).  Normalization happens once on the assembled xo via a small
    selection matmul that broadcasts per-head reciprocals.
  - GEMMs run in bf16 (measured ~2x faster than fp32r per matmul); PSUM
    accumulation is fp32.  The softmax-denominator reciprocal path stays
    float32r so the normalization factor is near-exact.
"""

import numpy as np

B, S, C = 8, 512, 1024
H, HD, R, HID = 16, 64, 32, 1024
NC3 = 3 * C
NCORES = 8
KC = C // 128          # 8 contraction chunks
MQK = 2 * C // 128     # 16 q+k channel-major output chunks
MASK_NEG = -50.0
VSTRIDE = HD + 1       # v columns per head incl. ones column

_cache = {}


def _get_nc():
    if "nc" in _cache:
        return _cache["nc"]

    from contextlib import ExitStack
    import concourse.tile as tile
    from concourse import bacc, mybir

    f32 = mybir.dt.float32
    f32r = mybir.dt.float32r
    bf16 = mybir.dt.bfloat16
    AF = mybir.ActivationFunctionType
    ALU = mybir.AluOpType

    nc = bacc.Bacc("TRN2", target_bir_lowering=False, debug=False)

    def din(name, shape, dt=bf16):
        return nc.dram_tensor(name, list(shape), dt, kind="ExternalInput")

    xT_d = din("xT", (C, S))
    maskb_d = din("maskb", (128, 4), f32)
    sel_d = din("sel", (H, C), f32r)
    qkv_w_d = din("qkv_w", (C, NC3))
    qkv_la_d = din("qkv_la", (C, R))
    qkv_lb_d = din("qkv_lb", (R, NC3))
    proj_w_d = din("proj_w", (C, C))
    proj_b_d = din("proj_b", (C,), f32)
    proj_la_d = din("proj_la", (C, R))
    proj_lb_d = din("proj_lb", (R, C))
    fc1_w_d = din("fc1_w", (C, HID))
    fc1_b_d = din("fc1_b", (HID,), f32)
    fc1_la_d = din("fc1_la", (C, R))
    fc1_lb_d = din("fc1_lb", (R, HID))
    fc2_w_d = din("fc2_w", (HID, C))
    fc2_b_d = din("fc2_b", (C,), f32)
    fc2_la_d = din("fc2_la", (HID, R))
    fc2_lb_d = din("fc2_lb", (R, C))
    outT_d = nc.dram_tensor("outT", [C, S], f32, kind="ExternalOutput")

    with tile.TileContext(nc) as tc, ExitStack() as ctx:
        resident = ctx.enter_context(tc.tile_pool(name="resident", bufs=1))
        wpool = ctx.enter_context(tc.tile_pool(name="wstream", bufs=4))
        psum = ctx.enter_context(tc.tile_pool(name="psum", bufs=8, space="PSUM"))
        expp = ctx.enter_context(tc.tile_pool(name="expp", bufs=2))
        tmpp = ctx.enter_context(tc.tile_pool(name="tmpp", bufs=2))
        outp = ctx.enter_context(tc.tile_pool(name="outp", bufs=3))

        # ---- resident loads -------------------------------------------------
        xT = resident.tile([128, KC, S], bf16, name="xT", tag="xT")
        nc.sync.dma_start(xT[:], xT_d[:].rearrange("(c p) s -> p c s", p=128))
        maskb = resident.tile([128, 4], f32, name="maskb", tag="maskb")
        nc.sync.dma_start(maskb[:], maskb_d[:])
        sel = resident.tile([H, C], f32r, name="sel", tag="sel")
        nc.sync.dma_start(sel[:], sel_d[:])

        la = {}
        lb = {}
        for nm, la_d, lb_d, ncols in (
            ("qkv", qkv_la_d, qkv_lb_d, NC3),
            ("fc1", fc1_la_d, fc1_lb_d, HID),
            ("fc2", fc2_la_d, fc2_lb_d, C),
            ("proj", proj_la_d, proj_lb_d, C),
        ):
            la[nm] = resident.tile(
                [128, KC, R], bf16, name=f"la_{nm}", tag=f"la_{nm}"
            )
            nc.sync.dma_start(
                la[nm][:], la_d[:].rearrange("(c p) r -> p c r", p=128)
            )
            lb[nm] = resident.tile(
                [R, ncols], bf16, name=f"lb_{nm}", tag=f"lb_{nm}"
            )
            nc.sync.dma_start(lb[nm][:], lb_d[:])

        biases = {}
        for nm, b_d in (("fc1", fc1_b_d), ("fc2", fc2_b_d), ("proj", proj_b_d)):
            biases[nm] = resident.tile(
                [128, KC], f32, name=f"b_{nm}", tag=f"b_{nm}"
            )
            nc.sync.dma_start(
                biases[nm][:], b_d[:].rearrange("(m p) -> p m", p=128)
            )

        qkv_w_r = qkv_w_d[:].rearrange("(k p) n -> k p n", p=128)
        fc1_w_r = fc1_w_d[:].rearrange("(k p) n -> k p n", p=128)
        fc2_w_r = fc2_w_d[:].rearrange("(k p) n -> k p n", p=128)
        proj_w_r = proj_w_d[:].rearrange("(k p) n -> k p n", p=128)

        def lora_tT(nm, act):
            """tT = (act_rowmajor @ la)^T as a [R, S] tile; act is [128, KC, S]."""
            pt = psum.tile([128, S], f32, name=f"pt_{nm}", tag="psum")
            for kc in range(KC):
                nc.tensor.matmul(
                    pt[0:R, :], la[nm][:, kc, :], act[:, kc, :],
                    start=(kc == 0), stop=(kc == KC - 1),
                )
            t = resident.tile([R, S], bf16, name=f"tT_{nm}", tag=f"tT_{nm}")
            nc.any.tensor_copy(t[:], pt[0:R, :])
            return t

        # ---- qkv GEMM -------------------------------------------------------
        tT_qkv = lora_tT("qkv", xT)

        # q,k channel-major: qkT[:, m, :], m in [0,16) covers channels [0,2C)
        qkT = resident.tile([128, MQK, S], bf16, name="qkT", tag="qkT")
        for g in range(4):            # groups of 4 output chunks
            ps = [
                psum.tile([128, S], f32, name=f"ps{i}", tag="psum")
                for i in range(4)
            ]
            for kc in range(KC):
                wt = wpool.tile([128, 512], bf16, tag="w")
                nc.sync.dma_start(
                    wt[:], qkv_w_r[kc, :, g * 512:(g + 1) * 512]
                )
                for i in range(4):
                    nc.tensor.matmul(
                        ps[i][:], wt[:, i * 128:(i + 1) * 128],
                        xT[:, kc, :], start=(kc == 0), stop=False,
                    )
            for i in range(4):
                m = g * 4 + i
                nc.tensor.matmul(
                    ps[i][:], lb["qkv"][:, m * 128:(m + 1) * 128],
                    tT_qkv[:], start=False, stop=True,
                )
                nc.any.tensor_copy(qkT[:, m, :], ps[i][:])

        # v token-major with interleaved ones columns: v[:, sq, h*65:+64]
        v = resident.tile([128, 4, H * VSTRIDE], bf16, name="vtok", tag="vtok")
        for h in range(H):
            nc.vector.memset(
                v[:, :, h * VSTRIDE + HD:h * VSTRIDE + HD + 1], 1.0
            )
        for n in range(2):
            ps = [
                psum.tile([128, S], f32, name=f"psv{sq}", tag="psum")
                for sq in range(4)
            ]
            for kc in range(KC):
                wt = wpool.tile([128, 512], bf16, tag="w")
                nc.sync.dma_start(
                    wt[:], qkv_w_r[kc, :, 2 * C + n * 512:2 * C + (n + 1) * 512]
                )
                for sq in range(4):
                    nc.tensor.matmul(
                        ps[sq][:], xT[:, kc, sq * 128:(sq + 1) * 128],
                        wt[:], start=(kc == 0), stop=False,
                    )
            for sq in range(4):
                nc.tensor.matmul(
                    ps[sq][:], tT_qkv[:, sq * 128:(sq + 1) * 128],
                    lb["qkv"][:, 2 * C + n * 512:2 * C + (n + 1) * 512],
                    start=False, stop=True,
                )
                for hl in range(8):
                    h = n * 8 + hl
                    nc.vector.tensor_copy(
                        v[:, sq, h * VSTRIDE:h * VSTRIDE + HD],
                        ps[sq][:, hl * HD:(hl + 1) * HD],
                    )

        # ---- attention ------------------------------------------------------
        # xou: unnormalized attention output, channel-major [128, KC, S]
        xou = resident.tile([128, KC, S], bf16, name="xou", tag="xou")
        den = resident.tile([H, S], f32r, name="den", tag="den")
        for h in range(H):
            j, half = h // 2, h % 2
            p0 = 64 * half
            exp_t = expp.tile([128, 4, S], bf16, name="exp_t", tag="exp")
            for c in range(4):
                pa = psum.tile([128, S], f32, name="pa", tag="psum")
                nc.tensor.matmul(
                    pa[:],
                    qkT[p0:p0 + 64, 8 + j, c * 128:(c + 1) * 128],
                    qkT[p0:p0 + 64, j, :],
                )
                # exp(attn/8 + maskbias); masked keys -> exp(-50+eps) ~ 0
                nc.scalar.activation(
                    exp_t[:, c, :], pa[:], AF.Exp,
                    bias=maskb[:, c:c + 1], scale=0.125,
                )
            po = psum.tile([128, S], f32, name="po", tag="psum")
            for c in range(4):
                nc.tensor.matmul(
                    po[0:VSTRIDE, :],
                    v[:, c, h * VSTRIDE:(h + 1) * VSTRIDE],
                    exp_t[:, c, :],
                    start=(c == 0), stop=(c == 3),
                )
            tmd = tmpp.tile([128, S], f32r, name="tmd", tag="tmpd")
            nc.vector.tensor_copy(tmd[HD:HD + 1, :], po[HD:HD + 1, :])
            nc.sync.dma_start(den[h:h + 1, :], tmd[HD:HD + 1, :])
            if half == 0:
                nc.vector.tensor_copy(xou[0:64, j, :], po[0:HD, :])
            else:
                tmb = tmpp.tile([128, S], bf16, name="tmb", tag="tmpb")
                nc.vector.tensor_copy(tmb[0:HD, :], po[0:HD, :])
                nc.sync.dma_start(xou[64:128, j, :], tmb[0:HD, :])

        recip = resident.tile([H, S], f32r, name="recip", tag="recip")
        with nc.allow_low_precision(reason="f32r keeps full fp32 bits here"):
            nc.vector.reciprocal(recip[:], den[:])
        for j in range(KC):
            pn = psum.tile([128, S], f32, name="pn", tag="psum")
            nc.tensor.matmul(
                pn[:], sel[:, j * 128:(j + 1) * 128], recip[:]
            )
            nc.vector.tensor_mul(xou[:, j, :], xou[:, j, :], pn[:])
        xoT = xou  # normalized in place

        # ---- MLP fc1 + gelu -------------------------------------------------
        tT_fc1 = lora_tT("fc1", xoT)
        gT = resident.tile([128, KC, S], bf16, name="gT", tag="gT")
        for g in range(2):
            ps = [
                psum.tile([128, S], f32, name=f"psf{i}", tag="psum")
                for i in range(4)
            ]
            for kc in range(KC):
                wt = wpool.tile([128, 512], bf16, tag="w")
                nc.sync.dma_start(wt[:], fc1_w_r[kc, :, g * 512:(g + 1) * 512])
                for i in range(4):
                    nc.tensor.matmul(
                        ps[i][:], wt[:, i * 128:(i + 1) * 128],
                        xoT[:, kc, :], start=(kc == 0), stop=False,
                    )
            for i in range(4):
                m = g * 4 + i
                nc.tensor.matmul(
                    ps[i][:], lb["fc1"][:, m * 128:(m + 1) * 128],
                    tT_fc1[:], start=False, stop=True,
                )
                nc.scalar.activation(
                    gT[:, m, :], ps[i][:], AF.Gelu,
                    bias=biases["fc1"][:, m:m + 1],
                )

        # ---- MLP fc2 + residual --------------------------------------------
        tT_fc2 = lora_tT("fc2", gT)
        xo2T = resident.tile([128, KC, S], bf16, name="xo2T", tag="xo2T")
        for g in range(2):
            ps = [
                psum.tile([128, S], f32, name=f"psg{i}", tag="psum")
                for i in range(4)
            ]
            for kc in range(KC):
                wt = wpool.tile([128, 512], bf16, tag="w")
                nc.sync.dma_start(wt[:], fc2_w_r[kc, :, g * 512:(g + 1) * 512])
                for i in range(4):
                    nc.tensor.matmul(
                        ps[i][:], wt[:, i * 128:(i + 1) * 128],
                        gT[:, kc, :], start=(kc == 0), stop=False,
                    )
            for i in range(4):
                m = g * 4 + i
                nc.tensor.matmul(
                    ps[i][:], lb["fc2"][:, m * 128:(m + 1) * 128],
                    tT_fc2[:], start=False, stop=True,
                )
                # xo2 = (fc2_psum + bias) + xo  (residual)
                nc.vector.scalar_tensor_tensor(
                    xo2T[:, m, :], ps[i][:], biases["fc2"][:, m:m + 1],
                    xoT[:, m, :], op0=ALU.add, op1=ALU.add,
                )

        # ---- proj -----------------------------------------------------------
        tT_proj = lora_tT("proj", xo2T)
        outT_r = outT_d[:].rearrange("(m p) s -> m p s", p=128)
        for g in range(2):
            ps = [
                psum.tile([128, S], f32, name=f"psp{i}", tag="psum")
                for i in range(4)
            ]
            for kc in range(KC):
                wt = wpool.tile([128, 512], bf16, tag="w")
                nc.sync.dma_start(wt[:], proj_w_r[kc, :, g * 512:(g + 1) * 512])
                for i in range(4):
                    nc.tensor.matmul(
                        ps[i][:], wt[:, i * 128:(i + 1) * 128],
                        xo2T[:, kc, :], start=(kc == 0), stop=False,
                    )
            for i in range(4):
                m = g * 4 + i
                nc.tensor.matmul(
                    ps[i][:], lb["proj"][:, m * 128:(m + 1) * 128],
                    tT_proj[:], start=False, stop=True,
                )
                ot = outp.tile([128, S], f32, name="ot", tag="out")
                nc.scalar.activation(
                    ot[:], ps[i][:], AF.Identity,
                    bias=biases["proj"][:, m:m + 1],
                )
                nc.sync.dma_start(outT_r[m], ot[:])

    nc.compile()
    _cache["nc"] = nc
    return nc


def _bf16(a):
    import ml_dtypes

    return np.asarray(a, dtype=np.float32).astype(ml_dtypes.bfloat16)


def _make_in_maps(inputs):
    x = np.asarray(inputs["x"], dtype=np.float32)
    mask = np.asarray(inputs["mask"])
    sel = np.zeros((H, C), dtype=np.float32)
    for h in range(H):
        sel[h, h * HD:(h + 1) * HD] = 1.0
    shared = {"sel": sel}
    for k in (
        "qkv_w", "qkv_la", "qkv_lb", "proj_w", "proj_la", "proj_lb",
        "fc1_w", "fc1_la", "fc1_lb", "fc2_w", "fc2_la", "fc2_lb",
    ):
        shared[k] = np.ascontiguousarray(_bf16(inputs[k]))
    for k in ("proj_b", "fc1_b", "fc2_b"):
        shared[k] = np.ascontiguousarray(inputs[k], dtype=np.float32)
    in_maps = []
    for b in range(NCORES):
        m = mask[b, :S].astype(bool)
        maskb = np.where(m, 0.0, MASK_NEG).astype(np.float32)
        in_maps.append(
            dict(
                shared,
                xT=np.ascontiguousarray(_bf16(x[b].T)),
                maskb=np.ascontiguousarray(maskb.reshape(4, 128).T),
            )
        )
    return in_maps


def _run(inputs, trace=False):
    from concourse.bass_utils import run_bass_kernel_spmd

    nc = _get_nc()
    in_maps = _make_in_maps(inputs)
    res = run_bass_kernel_spmd(nc, in_maps, list(range(NCORES)), trace=trace)
    out = np.stack(
        [np.ascontiguousarray(res.results[b]["outT"].T) for b in range(NCORES)]
    )
    return out, res


def kernel(**inputs):
    out, _ = _run(inputs, trace=False)
    return out
